# revision 4
# baseline (speedup 1.0000x reference)
"""Distributed Bass attention kernel for 8 TRN2 NeuronCores.

Problem: full-dim attention (no head split), x:(2,4096,2048), 4x 2048^2 weights.
Sharding: batch+sequence parallel. Core c owns batch b=c//4 and query rows
[1024*(c%4), 1024*(c%4+1)). Each core computes its local K^T/V shard, the
shards are AllGather-ed within the 4-core replica group of the same batch
(overlapped with the Q projection), then each core runs attention for its
1024 query rows and the output projection. Host reassembles rows.

All TensorEngine math in bf16 (fp32 PSUM accumulation). Softmax uses no max
subtraction (scores ~ N(0,1) by construction); 1/rowsum is folded into the
final PSUM->SBUF copy after the wo projection.
"""

import numpy as np
import ml_dtypes

BF16 = ml_dtypes.bfloat16

D = 2048          # model dim
S = 4096          # sequence length per batch
BATCH = 2
NCORES = 8
GROUP = 4         # replica group size (cores per batch)
ROWS = S // GROUP # query rows per core = 1024
P = 128           # partitions
DT = D // P       # 16 d-tiles
IT = ROWS // P    # 8 i-tiles per core
JT = S // P       # 32 j-tiles (full seq)
NG = 2            # i-tile groups of 4 (moving operand 4*128=512 wide)
GSZ = IT // NG    # 4 i-tiles per group
SCALE = 1.0 / float(np.sqrt(D))

_CACHE = {}


def _build():
    from concourse import bacc, mybir, tile
    from concourse.masks import make_identity

    f32 = mybir.dt.float32
    bf16 = mybir.dt.bfloat16

    nc = bacc.Bacc("TRN2", target_bir_lowering=False, debug=False,
                   num_devices=NCORES)

    xt_d = nc.dram_tensor("xt", [D, ROWS], bf16, kind="ExternalInput")
    wqt_d = nc.dram_tensor("wqt", [D, D], bf16, kind="ExternalInput")
    wkt_d = nc.dram_tensor("wkt", [D, D], bf16, kind="ExternalInput")
    wvt_d = nc.dram_tensor("wvt", [D, D], bf16, kind="ExternalInput")
    wot_d = nc.dram_tensor("wot", [D, D], bf16, kind="ExternalInput")
    out_d = nc.dram_tensor("out", [ROWS, D], f32, kind="ExternalOutput")

    RG = [[0, 1, 2, 3], [4, 5, 6, 7]]

    with tile.TileContext(nc) as tc:
        with (
            tc.tile_pool(name="dram", bufs=1, space="DRAM") as dram,
            tc.tile_pool(name="persist", bufs=1) as persist,
            tc.tile_pool(name="psum", bufs=2, space="PSUM") as psum,
            tc.tile_pool(name="io", bufs=2) as io,
        ):
            # DRAM bounce + gather buffers for the collectives
            kt_b = dram.tile([D, ROWS], bf16)
            v_b = dram.tile([ROWS, D], bf16)
            kt_g = dram.tile([GROUP, D, ROWS], bf16)
            v_g = dram.tile([GROUP, ROWS, D], bf16)

            # small persistent SBUF tensors
            linv_s = persist.tile([P, IT], f32)             # 1/rowsum per i-tile
            lparts = persist.tile([P, GSZ, S // 512], f32)  # partial rowsums
            ident = persist.tile([P, P], bf16)
            make_identity(nc, ident[:])

            with tc.tile_pool(name="qtpool", bufs=1) as qtpool:
                qt_s = qtpool.tile([P, DT, ROWS], bf16)     # q^T  [e, i]

                # ---------------- Phase 1: projections ----------------
                with tc.tile_pool(name="proj", bufs=2) as proj:
                    xt_s = proj.tile([P, DT, ROWS], bf16, bufs=1)
                    nc.sync.dma_start(
                        out=xt_s[:],
                        in_=xt_d[:, :].rearrange("(t p) i -> p t i", p=P))

                    # k^T then q^T: out[e-tile, i] = sum_d wt[d,e]^T x^T[d,i]
                    for w_d, is_k in ((wkt_d, True), (wqt_d, False)):
                        for et in range(DT):
                            wcol = proj.tile([P, DT, P], bf16, tag="wcol",
                                             bufs=3)
                            nc.sync.dma_start(
                                out=wcol[:],
                                in_=w_d[:, et * P:(et + 1) * P]
                                .rearrange("(t p) e -> p t e", p=P))
                            kt_t = proj.tile([P, ROWS], bf16, tag="kt_t",
                                             bufs=3)
                            for ic in range(ROWS // 512):
                                ps = psum.tile([P, 512], f32, tag="acc")
                                for dt_i in range(DT):
                                    nc.tensor.matmul(
                                        ps[:],
                                        wcol[:, dt_i, :],
                                        xt_s[:, dt_i, ic * 512:(ic + 1) * 512],
                                        start=(dt_i == 0),
                                        stop=(dt_i == DT - 1))
                                dst = (kt_t[:, ic * 512:(ic + 1) * 512]
                                       if is_k else
                                       qt_s[:, et, ic * 512:(ic + 1) * 512])
                                nc.scalar.copy(dst, ps[:])
                            if is_k:
                                nc.sync.dma_start(
                                    out=kt_b[et * P:(et + 1) * P, :],
                                    in_=kt_t[:])
                        if is_k:
                            # AllGather K^T (overlaps v/q projections)
                            nc.gpsimd.collective_compute(
                                "AllGather", mybir.AluOpType.bypass,
                                replica_groups=RG,
                                ins=[kt_b.opt()], outs=[kt_g.opt()])
                            # v: out[j-tile, d'] = sum_d x^T[d,j]^T wvt[d,d']
                            for qd in range(4):
                                wvq = proj.tile([P, DT, 512], bf16, tag="wq4",
                                                bufs=2)
                                nc.sync.dma_start(
                                    out=wvq[:],
                                    in_=wvt_d[:, qd * 512:(qd + 1) * 512]
                                    .rearrange("(t p) d -> p t d", p=P))
                                for jt in range(IT):
                                    ps = psum.tile([P, 512], f32, tag="acc")
                                    for dt_i in range(DT):
                                        nc.tensor.matmul(
                                            ps[:],
                                            xt_s[:, dt_i, jt * P:(jt + 1) * P],
                                            wvq[:, dt_i, :],
                                            start=(dt_i == 0),
                                            stop=(dt_i == DT - 1))
                                    v_t = proj.tile([P, 512], bf16, tag="v_t",
                                                    bufs=4)
                                    nc.scalar.copy(v_t[:], ps[:])
                                    nc.sync.dma_start(
                                        out=v_b[jt * P:(jt + 1) * P,
                                                qd * 512:(qd + 1) * 512],
                                        in_=v_t[:])
                            nc.gpsimd.collective_compute(
                                "AllGather", mybir.AluOpType.bypass,
                                replica_groups=RG,
                                ins=[v_b.opt()], outs=[v_g.opt()])

                # ------------- Phase 2: attention -------------
                with tc.tile_pool(name="cpool", bufs=1) as cpool:
                    ctxt_s = cpool.tile([P, DT, ROWS], bf16)   # ctx^T
                    pt_s = cpool.tile([P, JT, GSZ * P], bf16)  # P^T [j, gi]

                    with tc.tile_pool(name="attn", bufs=2) as attn:
                        for g in range(NG):
                            # --- A: scores + exp + transpose for group g ---
                            for c in range(S // 512):  # 8 key chunks of 512
                                r, half = c // 2, c % 2
                                kbuf = attn.tile([P, DT, 512], bf16,
                                                 tag="kbuf", bufs=2)
                                nc.sync.dma_start(
                                    out=kbuf[:],
                                    in_=kt_g[r, :,
                                             half * 512:(half + 1) * 512]
                                    .rearrange("(t p) j -> p t j", p=P))
                                for tg in range(GSZ):
                                    it = g * GSZ + tg
                                    sps = psum.tile([P, 512], f32,
                                                    tag="scores")
                                    for e in range(DT):
                                        nc.tensor.matmul(
                                            sps[:],
                                            qt_s[:, e, it * P:(it + 1) * P],
                                            kbuf[:, e, :],
                                            start=(e == 0),
                                            stop=(e == DT - 1))
                                    pstage = attn.tile([P, 512], bf16,
                                                       tag="pstage", bufs=4)
                                    nc.scalar.activation(
                                        pstage[:], sps[:],
                                        mybir.ActivationFunctionType.Exp,
                                        scale=SCALE,
                                        accum_out=lparts[:, tg, c:c + 1])
                                    for b in range(4):
                                        tps = psum.tile([P, P], bf16,
                                                        tag="tr")
                                        nc.tensor.transpose(
                                            tps[:],
                                            pstage[:, b * P:(b + 1) * P],
                                            ident[:])
                                        nc.vector.tensor_copy(
                                            pt_s[:, c * 4 + b,
                                                 tg * P:(tg + 1) * P],
                                            tps[:])
                            # rowsums -> 1/l
                            for tg in range(GSZ):
                                it = g * GSZ + tg
                                lsum = attn.tile([P, 1], f32, tag="lsum",
                                                 bufs=4)
                                nc.vector.tensor_reduce(
                                    lsum[:], lparts[:, tg, :],
                                    axis=mybir.AxisListType.X,
                                    op=mybir.AluOpType.add)
                                nc.vector.reciprocal(
                                    linv_s[:, it:it + 1], lsum[:])
                            # --- B: ctx^T[d', gi] = sum_j V[j,d']^T P^T ---
                            for dp2 in range(DT // 2):  # pairs of d'-tiles
                                vcol = attn.tile([P, JT, 256], bf16,
                                                 tag="vcol", bufs=2)
                                for r in range(GROUP):
                                    nc.sync.dma_start(
                                        out=vcol[:, r * IT:(r + 1) * IT, :],
                                        in_=v_g[r, :,
                                                dp2 * 256:(dp2 + 1) * 256]
                                        .rearrange("(t p) d -> p t d", p=P))
                                for ds in range(2):
                                    dp = dp2 * 2 + ds
                                    cps = psum.tile([P, 512], f32, tag="ctx")
                                    for jt in range(JT):
                                        nc.tensor.matmul(
                                            cps[:],
                                            vcol[:, jt, ds * P:(ds + 1) * P],
                                            pt_s[:, jt, :],
                                            start=(jt == 0),
                                            stop=(jt == JT - 1))
                                    nc.vector.tensor_copy(
                                        ctxt_s[:, dp, g * 512:(g + 1) * 512],
                                        cps[:])

                    # ------------- Phase 3: output projection -------------
                    with tc.tile_pool(name="oproj", bufs=2) as oproj:
                        for fq in range(4):
                            woq = oproj.tile([P, DT, 512], bf16, tag="woq",
                                             bufs=2)
                            nc.sync.dma_start(
                                out=woq[:],
                                in_=wot_d[:, fq * 512:(fq + 1) * 512]
                                .rearrange("(t p) f -> p t f", p=P))
                            for it in range(IT):
                                ops = psum.tile([P, 512], f32, tag="acc")
                                for dp in range(DT):
                                    nc.tensor.matmul(
                                        ops[:],
                                        ctxt_s[:, dp, it * P:(it + 1) * P],
                                        woq[:, dp, :],
                                        start=(dp == 0), stop=(dp == DT - 1))
                                osb = io.tile([P, 512], f32, tag="osb",
                                              bufs=3)
                                nc.scalar.mul(osb[:], ops[:],
                                              linv_s[:, it:it + 1])
                                nc.sync.dma_start(
                                    out=out_d[it * P:(it + 1) * P,
                                              fq * 512:(fq + 1) * 512],
                                    in_=osb[:])

    nc.compile()
    return nc


def _get_nc():
    if "nc" not in _CACHE:
        _CACHE["nc"] = _build()
    return _CACHE["nc"]


def _in_maps(x, wq, wk, wv, wo):
    wqt = np.ascontiguousarray(np.asarray(wq, np.float32).T).astype(BF16)
    wkt = np.ascontiguousarray(np.asarray(wk, np.float32).T).astype(BF16)
    wvt = np.ascontiguousarray(np.asarray(wv, np.float32).T).astype(BF16)
    wot = np.ascontiguousarray(np.asarray(wo, np.float32).T).astype(BF16)
    x = np.asarray(x, np.float32)
    maps = []
    for c in range(NCORES):
        b, r = c // GROUP, c % GROUP
        xt = np.ascontiguousarray(
            x[b, r * ROWS:(r + 1) * ROWS, :].T).astype(BF16)
        maps.append({"xt": xt, "wqt": wqt, "wkt": wkt, "wvt": wvt,
                     "wot": wot})
    return maps


def run(x, wq, wk, wv, wo, trace=False, **trace_kwargs):
    from concourse.bass_utils import run_bass_kernel_spmd
    nc = _get_nc()
    res = run_bass_kernel_spmd(nc, _in_maps(x, wq, wk, wv, wo),
                               list(range(NCORES)), trace=trace,
                               **trace_kwargs)
    out = np.empty((BATCH, S, D), np.float32)
    for c in range(NCORES):
        b, r = c // GROUP, c % GROUP
        out[b, r * ROWS:(r + 1) * ROWS, :] = res.results[c]["out"]
    return out, res


def kernel(x, wq, wk, wv, wo):
    out, _ = run(x, wq, wk, wv, wo)
    return out
